# revision 32
# baseline (speedup 1.0000x reference)
"""GAT-style masked-softmax attention kernel for Trainium2 (8 NeuronCores).

Problem (per batch b of 32):
    e   = leaky_relu(h @ a1 + (g @ a2)^T, 0.2)        # (N, M)
    att = softmax(where(adj > 0, e, -9e15), axis=-1)  # (N, M)
    out = (att * adj.sum(-1, keepdims=True)) @ g      # (N, D)

Strategy (pure data parallel over B=32 -> 4 batches/core):
  * No row-max subtraction: e is bounded (|e| <= ~20), exp() is fp32-safe, and
    softmax is shift-invariant, so s = adj * exp(e) and
    out_i = (deg_i / rowsum_i) * (s @ g)_i.
  * Scores live TRANSPOSED (j on partitions). leaky_relu(u_i + v_j) is ONE
    ACT pass: Prelu(alpha=0.2) with per-partition bias = v (v_j is
    per-partition in this layout); u is broadcast along partitions by a K=1
    ones matmul into PSUM. Then one Exp pass (same act-table set).
  * adj int32 -> bf16 on DVE, transposed on-chip by the DMA xbar.
    deg_i rides a ones-matmul over the transposed mask (replicated row form).
  * Second matmul runs weight-stationary: lhsT = g (one LDWEIGHTS per j
    block), rhs = s^T slices (F=512) -> out^T (d on partitions) in PSUM.
    Output is written to DRAM transposed; the host un-transposes (layout
    only, no math). rowsum rides a ones-matmul reusing the deg PSUM; the
    deg/rowsum scale happens row-wise while copying out^T off PSUM.
  * DMA queues: sync(HWDGE) = h/g + adj loads + xbar transposes (one batch
    prefetched ahead); gpsimd(SWDGE) = u bounce + output stores; the scalar
    queue carries activations only. One-batch lookahead on BOTH the adj
    stage and the u/v projection stage keeps the mask and activation
    pipelines a full batch ahead of the matmul pipeline.

Self-contained: hardcodes shapes B,N,M,D = 32,1024,1024,128 on 8 cores.
"""

import sys

if "/opt/trn_rl_repo" not in sys.path:
    sys.path.insert(0, "/opt/trn_rl_repo")

import numpy as np

import concourse.bacc as bacc
import concourse.mybir as mybir
import concourse.tile as tile
import concourse.bass_utils as bass_utils

F32 = mybir.dt.float32
BF16 = mybir.dt.bfloat16
I32 = mybir.dt.int32
OP = mybir.AluOpType
AF = mybir.ActivationFunctionType

B, N, M, D = 32, 1024, 1024, 128
NCORES = 8
BPC = B // NCORES  # batches per core
NI = N // 128      # i blocks
NJ = M // 128      # j blocks


def build_bass():
    nc = bacc.Bacc("TRN2", target_bir_lowering=False, debug=False)

    h_in = nc.dram_tensor("input1", [BPC, N, D], F32, kind="ExternalInput").ap()
    g_in = nc.dram_tensor("input2", [BPC, M, D], F32, kind="ExternalInput").ap()
    adj_in = nc.dram_tensor("adj", [BPC, N, M], I32, kind="ExternalInput").ap()
    a1_in = nc.dram_tensor("a1", [D, 1], F32, kind="ExternalInput").ap()
    a2_in = nc.dram_tensor("a2", [D, 1], F32, kind="ExternalInput").ap()
    # out^T: host transposes (0,2,1) after gather
    out_d = nc.dram_tensor("out", [BPC, D, N], F32, kind="ExternalOutput").ap()

    urow_scr = nc.dram_tensor("urow_scr", [BPC, N], F32).ap()

    with tile.TileContext(nc) as tc:
        with (
            tc.tile_pool(name="singles", bufs=1) as singles,
            tc.tile_pool(name="hg", bufs=2) as hg_pool,
            tc.tile_pool(name="gbf", bufs=2) as gbf_pool,
            tc.tile_pool(name="adjp", bufs=7) as adj_pool,
            tc.tile_pool(name="afp", bufs=4) as af_pool,
            tc.tile_pool(name="aft", bufs=2) as aft_pool,
            tc.tile_pool(name="plp", bufs=2) as pl_pool,
            tc.tile_pool(name="exp", bufs=2) as ex_pool,
            tc.tile_pool(name="st", bufs=2) as st_pool,
            tc.tile_pool(name="small", bufs=4) as small,
            tc.tile_pool(name="rows", bufs=2) as rows_pool,
            tc.tile_pool(name="rows1", bufs=1) as rows1_pool,
            tc.tile_pool(name="psUT", bufs=2, space="PSUM") as psUT,
            tc.tile_pool(name="psC", bufs=2, space="PSUM") as psC,
        ):
            # ---- static prep ------------------------------------------------
            ones_row = singles.tile([1, max(N, M)], F32)
            nc.vector.memset(ones_row[:], 1.0)
            ones_sq_bf = singles.tile([128, 128], BF16)
            nc.vector.memset(ones_sq_bf[:], 1.0)

            a1row = singles.tile([1, D], F32)
            nc.gpsimd.dma_start(a1row[:], a1_in.transpose((1, 0)))
            a2row = singles.tile([1, D], F32)
            nc.gpsimd.dma_start(a2row[:], a2_in.transpose((1, 0)))

            a1bc = singles.tile([128, D], F32)
            a2bc = singles.tile([128, D], F32)
            bc_ps = psC.tile([128, N], F32, tag="combo")
            nc.tensor.matmul(
                bc_ps[:, 0:D], ones_row[:, :128], a1row[:], start=True, stop=True
            )
            nc.tensor.matmul(
                bc_ps[:, D : 2 * D], ones_row[:, :128], a2row[:], start=True, stop=True
            )
            nc.vector.tensor_copy(a1bc[:], bc_ps[:, 0:D])
            nc.vector.tensor_copy(a2bc[:], bc_ps[:, D : 2 * D])

            def prefetch(b):
                h_t = hg_pool.tile([128, NI, D], F32, tag="h")
                nc.sync.dma_start(
                    h_t[:], h_in[b].rearrange("(ib p) d -> p ib d", p=128)
                )
                g_t = hg_pool.tile([128, NJ, D], F32, tag="g")
                nc.sync.dma_start(
                    g_t[:], g_in[b].rearrange("(jb p) d -> p jb d", p=128)
                )
                adj_ts = []
                for ib in range(NI):
                    adj_t = adj_pool.tile([128, M], I32)
                    nc.sync.dma_start(
                        adj_t[:], adj_in[b, ib * 128 : (ib + 1) * 128, :]
                    )
                    adj_ts.append(adj_t)
                return h_t, g_t, adj_ts

            def stage_adj(adj_ts):
                # cast to bf16, xbar transpose:
                # afT[p, ib, jb, i'] = adj[b, ib*128+i', jb*128+p]
                afT = aft_pool.tile([128, NI, NJ, 128], BF16)
                for ib in range(NI):
                    af = af_pool.tile([128, M], BF16)
                    nc.vector.tensor_scalar(af[:], adj_ts[ib][:], 1.0, None, OP.mult)
                    nc.sync.dma_start_transpose(afT[:, ib], af[:])
                return afT

            def stage_uv(b, h_t, g_t):
                # bf16 g for the output matmul
                g_bf = gbf_pool.tile([128, NJ, D], BF16)
                nc.vector.tensor_copy(g_bf[:], g_t[:])
                # u/v projections (u_i = h_i . a1, v_j = g_j . a2)
                ucols = small.tile([128, NI], F32, tag="ucols")
                vcols = small.tile([128, NJ], F32, tag="vcols")
                uscr = small.tile([128, D], F32, tag="uscr")
                for ib in range(NI):
                    nc.vector.scalar_tensor_tensor(
                        uscr[:], h_t[:, ib, :], 0.0, a1bc[:],
                        OP.bypass, OP.mult, accum_out=ucols[:, ib : ib + 1],
                    )
                for jb in range(NJ):
                    nc.vector.scalar_tensor_tensor(
                        uscr[:], g_t[:, jb, :], 0.0, a2bc[:],
                        OP.bypass, OP.mult, accum_out=vcols[:, jb : jb + 1],
                    )
                # u columns -> one row vector, via a DRAM bounce
                nc.gpsimd.dma_start(
                    urow_scr[b].rearrange("(ib p) -> p ib", p=128), ucols[:]
                )
                urow = rows_pool.tile([1, N], F32, tag="urow")
                nc.gpsimd.dma_start(urow[:], urow_scr[b].unsqueeze(0))
                # u broadcast along partitions
                u_ps = psUT.tile([128, N], F32, tag="ut")
                for half in range(2):
                    fs = slice(half * 512, (half + 1) * 512)
                    nc.tensor.matmul(
                        u_ps[:, fs], ones_row[0:1, 0:128], urow[0:1, fs],
                        start=True, stop=True,
                    )
                return g_bf, vcols, u_ps

            def stage_deg(afT):
                # deg_i = sum_j adj[i, j] (replicated rows)
                combo_ps = psC.tile([128, N], F32, tag="combo")
                for jb in range(NJ):
                    for half in range(2):
                        ibs = slice(half * 4, (half + 1) * 4)
                        fs = slice(half * 512, (half + 1) * 512)
                        nc.tensor.matmul(
                            combo_ps[:, fs], ones_sq_bf[:], afT[:, ibs, jb, :],
                            start=(jb == 0), stop=(jb == NJ - 1),
                        )
                degrow = rows_pool.tile([128, N], F32, tag="degrow")
                nc.vector.tensor_copy(degrow[:], combo_ps[:])
                return combo_ps, degrow

            pf = prefetch(0)
            afT = stage_adj(pf[2])
            uv = stage_uv(0, pf[0], pf[1])
            cd = stage_deg(afT)
            for b in range(BPC):
                g_bf, vcols, u_ps = uv
                combo_ps, degrow = cd

                # next batch: bulk loads + adj + u/v + deg stages, emitted
                # before this batch's masks so the tensor engine always has
                # ready work during this batch's activation phase
                if b + 1 < BPC:
                    pf = prefetch(b + 1)
                    afT_next = stage_adj(pf[2])
                    uv_next = stage_uv(b + 1, pf[0], pf[1])
                    cd_next = stage_deg(afT_next)
                sT = st_pool.tile([128, NJ, N], BF16)
                pl = None
                for jb in range(NJ):
                    if jb % 4 == 0:
                        pl = pl_pool.tile([128, 4, N], F32)
                    nc.scalar.activation(
                        pl[:, jb % 4, :], u_ps[:], AF.Prelu,
                        bias=vcols[:, jb : jb + 1], alpha=0.2,
                    )
                    if jb % 4 == 3:
                        ex = ex_pool.tile([128, 4, N], BF16)
                        nc.scalar.activation(ex[:], pl[:], AF.Exp)
                        for j2 in (jb - 3, jb - 1):
                            nc.vector.tensor_tensor(
                                sT[:, j2 : j2 + 2, :].rearrange(
                                    "p a (ib c) -> p a ib c", ib=NI
                                ),
                                afT[:, :, j2 : j2 + 2, :].transpose((0, 2, 1, 3)),
                                ex[:, j2 % 4 : j2 % 4 + 2, :].rearrange(
                                    "p a (ib c) -> p a ib c", ib=NI
                                ),
                                OP.mult,
                            )

                # ---- out^T = g^T @ s^T (weight-stationary g) ----------------
                outT_ps = psUT.tile([128, N], F32, tag="ut")
                for jb in range(NJ):
                    for half in range(2):
                        fs = slice(half * 512, (half + 1) * 512)
                        nc.tensor.matmul(
                            outT_ps[:, fs], g_bf[:, jb, :], sT[:, jb, fs],
                            start=(jb == 0), stop=(jb == NJ - 1),
                        )

                # ---- rowsum_i = sum_j s^T[j, i] (reuses combo PSUM) ---------
                for jb in range(NJ):
                    for half in range(2):
                        fs = slice(half * 512, (half + 1) * 512)
                        nc.tensor.matmul(
                            combo_ps[:, fs], ones_sq_bf[:], sT[:, jb, fs],
                            start=(jb == 0), stop=(jb == NJ - 1),
                        )

                # ---- epilogue: out^T * (deg / rowsum), all row-form ---------
                rrow = rows1_pool.tile([128, N], F32, tag="rrow")
                nc.vector.reciprocal_approx_fast(rrow[:], combo_ps[:])
                fac = rows1_pool.tile([128, N], F32, tag="fac")
                nc.vector.tensor_tensor(fac[:], degrow[:], rrow[:], OP.mult)
                outsbT = rows1_pool.tile([128, N], F32, tag="outsbT")
                nc.vector.tensor_tensor(outsbT[:], outT_ps[:], fac[:], OP.mult)
                nc.gpsimd.dma_start(out_d[b], outsbT[:])
                if b + 1 < BPC:
                    afT = afT_next
                    uv = uv_next
                    cd = cd_next

    nc.compile()
    return nc


_CACHE = {}


def _get_nc():
    if "nc" not in _CACHE:
        _CACHE["nc"] = build_bass()
    return _CACHE["nc"]


def _make_in_maps(input1, input2, adj, a1, a2):
    input1 = np.ascontiguousarray(np.asarray(input1, dtype=np.float32))
    input2 = np.ascontiguousarray(np.asarray(input2, dtype=np.float32))
    adj = np.ascontiguousarray(np.asarray(adj, dtype=np.int32))
    a1 = np.ascontiguousarray(np.asarray(a1, dtype=np.float32))
    a2 = np.ascontiguousarray(np.asarray(a2, dtype=np.float32))
    in_maps = []
    for c in range(NCORES):
        sl = slice(c * BPC, (c + 1) * BPC)
        in_maps.append(
            {
                "input1": input1[sl],
                "input2": input2[sl],
                "adj": adj[sl],
                "a1": a1,
                "a2": a2,
            }
        )
    return in_maps


def _gather(res):
    # device emits out^T (BPC, D, N); un-transpose (layout only)
    return np.concatenate(
        [r["out"].transpose(0, 2, 1) for r in res.results], axis=0
    )


def kernel(input1, input2, adj, a1, a2):
    nc = _get_nc()
    res = bass_utils.run_bass_kernel_spmd(
        nc, _make_in_maps(input1, input2, adj, a1, a2),
        core_ids=list(range(NCORES)),
    )
    return _gather(res)


def run_traced(input1, input2, adj, a1, a2, trace_cores=None):
    nc = _get_nc()
    res = bass_utils.run_bass_kernel_spmd(
        nc, _make_in_maps(input1, input2, adj, a1, a2),
        core_ids=list(range(NCORES)),
        trace=True,
        trace_cores=trace_cores or [0],
    )
    return _gather(res), res
